# revision 8
# baseline (speedup 1.0000x reference)
"""DecoderLayer (spatial attn + causal temporal attn + FFN) on 8 TRN2 NeuronCores.

Sharding: data-parallel. Spatial attention shards T (16 t-slices/core);
an on-device AllToAll reshards to N (32 cells/core) for causal temporal
attention + FFN. Activations are kept feature-major ([d, token]) on chip so
every matmul consumes natural layouts; the host pre/post-transposes x/out.

Matmul dtypes: fp32r (full-rate fp32 at free-dim>=256) for projections/FFN,
bf16 for softmax weights and V (error ~3e-3 on the attention delta).
"""
import sys

sys.path.insert(0, "/opt/trn_rl_repo")
import numpy as np
import concourse.bass as bass
import concourse.mybir as mybir
import concourse.tile as tile
from concourse import bacc
from concourse import bass_utils
from concourse.masks import make_upper_triangular

F32 = mybir.dt.float32
F32R = mybir.dt.float32r
BF16 = mybir.dt.bfloat16
EXP = mybir.ActivationFunctionType.Exp
SQUARE = mybir.ActivationFunctionType.Square
RSQRT = mybir.ActivationFunctionType.Rsqrt
ADD = mybir.AluOpType.add
MULT = mybir.AluOpType.mult
MAX = mybir.AluOpType.max

T, N, D, H, DK, DFF = 128, 256, 512, 8, 64, 2048
NC = 8
TS = T // NC          # 16 t-slices per core (spatial)
NS = N // NC          # 32 cells per core (temporal)
TOK = TS * N          # 4096 tokens per core
NB = 8                # temporal batch: cells per batch
NBATCH = NS // NB     # 4 temporal batches
BTOK = T * NB         # 1024 tokens per temporal batch


def _mm(nc, out, lhsT, rhs, start, stop):
    nc.tensor.matmul(out, lhsT, rhs, start=start, stop=stop)


def _load_w(nc, sb, name, dram_ap, kdim, ndim, tag):
    """Weight [kdim, ndim] -> SBUF [128, (kdim//128)*ndim] f32r, chunk dc at
    cols [dc*ndim, (dc+1)*ndim)."""
    kc = kdim // 128
    t = sb.tile([128, kc * ndim], F32R, tag=tag)
    for dc in range(kc):
        nc.sync.dma_start(t[:, dc * ndim:(dc + 1) * ndim],
                          dram_ap[dc * 128:(dc + 1) * 128, :].bitcast(F32R))
    return t


def build():
    nc = bacc.Bacc("TRN2", target_bir_lowering=False, debug=False,
                   num_devices=NC)

    def inp(name, shape):
        return nc.dram_tensor(name, shape, F32, kind="ExternalInput").ap()

    xs = inp("xs", [D, TOK])                       # spatial shard, feature-major
    w = {}
    for p in ("s", "t"):
        for m in ("q", "k", "v", "o"):
            w[p + m + "w"] = inp(p + m + "w", [D, D])
        for m in ("q", "k", "o"):
            w[p + m + "b"] = inp(p + m + "b", [128, 4])
        w[p + "vbb"] = inp(p + "vbb", [128, D])    # v bias broadcast to 128 rows
    f1w = inp("f1w", [D, DFF])
    f1b = inp("f1b", [128, DFF // 128])
    f2w = inp("f2w", [DFF, D])
    f2b = inp("f2b", [128, 4])
    lng = inp("lng", [128, 4])
    lnb = inp("lnb", [128, 4])
    ones1 = inp("ones1", [1, 128])
    ones128 = inp("ones128", [128, 1])
    out = nc.dram_tensor("out", [D, TOK], F32, kind="ExternalOutput").ap()

    with tile.TileContext(nc) as tc, \
         nc.allow_low_precision(reason="bf16 softmax weights and values"):
        with tc.tile_pool(name="const", bufs=1) as cb, \
             tc.tile_pool(name="dram", bufs=1, space="DRAM") as dram:
            ones_bf = cb.tile([128, 1], BF16, tag="obf")
            nc.vector.memset(ones_bf[:], 1.0)
            ones_r1 = cb.tile([1, 128], F32R, tag="o1r")
            nc.sync.dma_start(ones_r1[:], ones1.bitcast(F32R))
            ones_r128 = cb.tile([128, 1], F32R, tag="o128r")
            nc.sync.dma_start(ones_r128[:], ones128.bitcast(F32R))
            mask = cb.tile([128, 128], BF16, tag="mask")
            make_upper_triangular(nc, mask[:], val=1.0, diag=True)

            # a2a buffers: [dst, tloc(8), dc, p, n(32)] per half
            half_shape = [NC, TS // 2, 4, 128, 32]
            sends = [dram.tile(half_shape, F32, name=f"send{i}", tag=f"send{i}")
                     for i in range(2)]
            recvs = [dram.tile(half_shape, F32, name=f"recv{i}", tag=f"recv{i}")
                     for i in range(2)]
            r2d = dram.tile([4, 128, TOK], F32, tag="r2d")

            # ---------------- Phase A: spatial attention over n (shard T) ---
            with tc.tile_pool(name="wA", bufs=1) as wp, \
                 tc.tile_pool(name="actA", bufs=2) as ap, \
                 tc.tile_pool(name="smA", bufs=4) as sc, \
                 tc.tile_pool(name="psA", bufs=2, space="PSUM") as pp:
                wq = _load_w(nc, wp, "wq", w["sqw"], D, D, "wq")
                wk = _load_w(nc, wp, "wk", w["skw"], D, D, "wk")
                wv = _load_w(nc, wp, "wv", w["svw"], D, D, "wv")
                wo = _load_w(nc, wp, "wo", w["sow"], D, D, "wo")
                bq = wp.tile([128, 4], F32, tag="bq")
                nc.sync.dma_start(bq[:], w["sqb"])
                bk = wp.tile([128, 4], F32, tag="bk")
                nc.sync.dma_start(bk[:], w["skb"])
                bo = wp.tile([128, 4], F32, tag="bo")
                nc.sync.dma_start(bo[:], w["sob"])
                bvb = wp.tile([128, D], F32, tag="bvb")
                nc.sync.dma_start(bvb[:], w["svbb"])

                for tp in range(TS // 2):   # 8 pairs of t-slices, 512 tok each
                    xt = ap.tile([128, 2048], F32R, tag="xt")
                    for dc in range(4):
                        nc.sync.dma_start(
                            xt[:, dc * 512:(dc + 1) * 512],
                            xs[dc * 128:(dc + 1) * 128,
                               tp * 512:(tp + 1) * 512].bitcast(F32R))
                    qt = ap.tile([128, 2048], F32R, tag="qt")
                    kt = ap.tile([128, 2048], F32R, tag="kt")
                    for mo in range(4):
                        qps = pp.tile([128, 512], F32, tag="pa")
                        kps = pp.tile([128, 512], F32, tag="pb")
                        for dc in range(4):
                            _mm(nc, qps[:],
                                wq[:, dc * D + mo * 128: dc * D + (mo + 1) * 128],
                                xt[:, dc * 512:(dc + 1) * 512], dc == 0, dc == 3)
                            _mm(nc, kps[:],
                                wk[:, dc * D + mo * 128: dc * D + (mo + 1) * 128],
                                xt[:, dc * 512:(dc + 1) * 512], dc == 0, dc == 3)
                        nc.vector.tensor_scalar_add(
                            qt[:, mo * 512:(mo + 1) * 512], qps[:], bq[:, mo:mo + 1])
                        nc.vector.tensor_scalar_add(
                            kt[:, mo * 512:(mo + 1) * 512], kps[:], bk[:, mo:mo + 1])
                    # V token-major, bf16: chunk c (128 tokens) at cols c*512..
                    vt = ap.tile([128, 2048], BF16, tag="vt")
                    for c in range(4):
                        vps = pp.tile([128, 512], F32, tag="pa")
                        for dc in range(4):
                            _mm(nc, vps[:],
                                xt[:, dc * 512 + c * 128: dc * 512 + (c + 1) * 128],
                                wv[:, dc * D:(dc + 1) * D], dc == 0, dc == 3)
                        nc.vector.tensor_add(vt[:, c * 512:(c + 1) * 512],
                                             vps[:], bvb[:])
                    ot = ap.tile([128, 2048], F32R, tag="ot")
                    for tl in range(2):
                        for h in range(H):
                            po, dch = (h % 2) * 64, h // 2
                            base = dch * 512 + tl * 256
                            qrhs = qt[po:po + 64, base:base + 256]
                            dps = pp.tile([1, 256], F32, tag="pd")
                            es_list = []
                            for kc in range(2):
                                sps = pp.tile([128, 256], F32, tag="pb")
                                _mm(nc, sps[:],
                                    kt[po:po + 64,
                                       base + kc * 128: base + (kc + 1) * 128],
                                    qrhs, True, True)
                                es = sc.tile([128, 256], BF16, tag="es")
                                nc.scalar.activation(es[:], sps[:], EXP)
                                _mm(nc, dps[:], ones_bf[:], es[:],
                                    kc == 0, kc == 1)
                                es_list.append(es)
                            rec = sc.tile([1, 256], F32R, tag="rec")
                            nc.vector.reciprocal(rec[:], dps[:])
                            bps = pp.tile([64, 256], F32, tag="pe")
                            _mm(nc, bps[:], ones_r1[:, :64], rec[:], True, True)
                            ops = pp.tile([64, 256], F32, tag="pa")
                            for kc in range(2):
                                _mm(nc, ops[:],
                                    vt[:, (2 * tl + kc) * 512 + h * 64:
                                       (2 * tl + kc) * 512 + (h + 1) * 64],
                                    es_list[kc][:], kc == 0, kc == 1)
                            osl = ot[po:po + 64, base:base + 256]
                            nc.scalar.copy(osl, ops[:])
                            nc.vector.tensor_mul(osl, osl, bps[:])
                    r1 = ap.tile([128, 2048], F32, tag="r1")
                    for mo in range(4):
                        rps = pp.tile([128, 512], F32, tag="pb")
                        for dc in range(4):
                            _mm(nc, rps[:],
                                wo[:, dc * D + mo * 128: dc * D + (mo + 1) * 128],
                                ot[:, dc * 512:(dc + 1) * 512], dc == 0, dc == 3)
                        nc.vector.scalar_tensor_tensor(
                            r1[:, mo * 512:(mo + 1) * 512], rps[:],
                            bo[:, mo:mo + 1],
                            xt[:, mo * 512:(mo + 1) * 512].bitcast(F32),
                            ADD, ADD)
                    # scatter to a2a send buffers
                    r1v = r1.rearrange("p (dc tl n) -> p tl dc n", dc=4, n=256)
                    for tl in range(2):
                        t = 2 * tp + tl
                        sh, tloc = t // 8, t % 8
                        for j in range(NC):
                            nc.sync.dma_start(
                                sends[sh][j, tloc].transpose([1, 0, 2]),
                                r1v[:, tl, :, j * 32:(j + 1) * 32])
                    if tp == 3:
                        nc.gpsimd.collective_compute(
                            "AllToAll", mybir.AluOpType.bypass,
                            replica_groups=[list(range(NC))],
                            ins=[sends[0][:].opt()], outs=[recvs[0][:].opt()])
                if True:
                    nc.gpsimd.collective_compute(
                        "AllToAll", mybir.AluOpType.bypass,
                        replica_groups=[list(range(NC))],
                        ins=[sends[1][:].opt()], outs=[recvs[1][:].opt()])

            # ---------------- Phase B: temporal attention over t (shard N) --
            with tc.tile_pool(name="wB", bufs=1) as wp, \
                 tc.tile_pool(name="actBx", bufs=2) as apx, \
                 tc.tile_pool(name="actB", bufs=1) as ap, \
                 tc.tile_pool(name="smB", bufs=4) as sc, \
                 tc.tile_pool(name="psB", bufs=2, space="PSUM") as pp:
                wq = _load_w(nc, wp, "wq", w["tqw"], D, D, "wq")
                wk = _load_w(nc, wp, "wk", w["tkw"], D, D, "wk")
                wv = _load_w(nc, wp, "wv", w["tvw"], D, D, "wv")
                wo = _load_w(nc, wp, "wo", w["tow"], D, D, "wo")
                bq = wp.tile([128, 4], F32, tag="bq")
                nc.sync.dma_start(bq[:], w["tqb"])
                bk = wp.tile([128, 4], F32, tag="bk")
                nc.sync.dma_start(bk[:], w["tkb"])
                bo = wp.tile([128, 4], F32, tag="bo")
                nc.sync.dma_start(bo[:], w["tob"])
                bvb = wp.tile([128, D], F32, tag="bvb")
                nc.sync.dma_start(bvb[:], w["tvbb"])

                for b in range(NBATCH):   # 8 cells per batch, 1024 tokens
                    xt = apx.tile([128, 4 * BTOK], F32R, tag="xt")
                    for src in range(NC):
                        for hf in range(2):
                            for dc in range(4):
                                nc.sync.dma_start(
                                    xt[:, dc * BTOK + src * 128 + hf * 64:
                                       dc * BTOK + src * 128 + hf * 64 + 64]
                                    .rearrange("p (tl n) -> p tl n", n=NB),
                                    recvs[hf][src, :, dc, :, b * NB:(b + 1) * NB]
                                    .transpose([1, 0, 2]).bitcast(F32R))
                    qt = ap.tile([128, 4 * BTOK], F32R, tag="qt")
                    kt = ap.tile([128, 4 * BTOK], F32R, tag="kt")
                    for mo in range(4):
                        for sl in range(2):
                            qps = pp.tile([128, 512], F32, tag="pa")
                            kps = pp.tile([128, 512], F32, tag="pb")
                            for dc in range(4):
                                _mm(nc, qps[:],
                                    wq[:, dc * D + mo * 128: dc * D + (mo + 1) * 128],
                                    xt[:, dc * BTOK + sl * 512:
                                       dc * BTOK + (sl + 1) * 512], dc == 0, dc == 3)
                                _mm(nc, kps[:],
                                    wk[:, dc * D + mo * 128: dc * D + (mo + 1) * 128],
                                    xt[:, dc * BTOK + sl * 512:
                                       dc * BTOK + (sl + 1) * 512], dc == 0, dc == 3)
                            nc.vector.tensor_scalar_add(
                                qt[:, mo * BTOK + sl * 512: mo * BTOK + (sl + 1) * 512],
                                qps[:], bq[:, mo:mo + 1])
                            nc.vector.tensor_scalar_add(
                                kt[:, mo * BTOK + sl * 512: mo * BTOK + (sl + 1) * 512],
                                kps[:], bk[:, mo:mo + 1])
                    vt = ap.tile([128, NB * 512], BF16, tag="vt")
                    for nl in range(NB):
                        vps = pp.tile([128, 512], F32, tag="pa")
                        for dc in range(4):
                            lhsT = xt[:, dc * BTOK:(dc + 1) * BTOK].rearrange(
                                "p (t n) -> p t n", n=NB)[:, :, nl]
                            _mm(nc, vps[:], lhsT, wv[:, dc * D:(dc + 1) * D],
                                dc == 0, dc == 3)
                        nc.vector.tensor_add(vt[:, nl * 512:(nl + 1) * 512],
                                             vps[:], bvb[:])
                    ot = ap.tile([128, 4 * BTOK], F32R, tag="ot")
                    for pr in range(NB // 2):
                        n1 = 2 * pr
                        for h in range(H):
                            po, dch = (h % 2) * 64, h // 2
                            qh = qt[:, dch * BTOK:(dch + 1) * BTOK].rearrange(
                                "p (t n) -> p n t", n=NB)
                            kh = kt[:, dch * BTOK:(dch + 1) * BTOK].rearrange(
                                "p (t n) -> p t n", n=NB)
                            qpair = qh[po:po + 64, n1:n1 + 2, :]
                            dps = pp.tile([1, 256], F32, tag="pd")
                            esms = []
                            for j in range(2):
                                sps = pp.tile([128, 256], F32, tag="pb")
                                _mm(nc, sps[:], kh[po:po + 64, :, n1 + j],
                                    qpair, True, True)
                                es = sc.tile([128, 128], BF16, tag="es")
                                nc.scalar.activation(
                                    es[:], sps[:, j * 128:(j + 1) * 128], EXP)
                                esm = sc.tile([128, 128], BF16, tag="esm")
                                nc.vector.tensor_mul(esm[:], es[:], mask[:])
                                _mm(nc, dps[:, j * 128:(j + 1) * 128],
                                    ones_bf[:], esm[:], True, True)
                                esms.append(esm)
                            rec = sc.tile([1, 256], F32R, tag="rec")
                            nc.vector.reciprocal(rec[:], dps[:])
                            bps = pp.tile([64, 256], F32, tag="pe")
                            _mm(nc, bps[:], ones_r1[:, :64], rec[:], True, True)
                            for j in range(2):
                                ops = pp.tile([64, 128], F32, tag="pa")
                                _mm(nc, ops[:],
                                    vt[:, (n1 + j) * 512 + h * 64:
                                       (n1 + j) * 512 + (h + 1) * 64],
                                    esms[j][:], True, True)
                                osl = ot[:, dch * BTOK:(dch + 1) * BTOK].rearrange(
                                    "p (t n) -> p t n", n=NB)[po:po + 64, :, n1 + j]
                                nc.scalar.copy(osl, ops[:])
                                nc.vector.tensor_mul(
                                    osl, osl, bps[:, j * 128:(j + 1) * 128])
                    r2 = ap.tile([128, 4 * BTOK], F32, tag="r2")
                    for mo in range(4):
                        for sl in range(2):
                            rps = pp.tile([128, 512], F32, tag="pb")
                            for dc in range(4):
                                _mm(nc, rps[:],
                                    wo[:, dc * D + mo * 128: dc * D + (mo + 1) * 128],
                                    ot[:, dc * BTOK + sl * 512:
                                       dc * BTOK + (sl + 1) * 512], dc == 0, dc == 3)
                            nc.vector.scalar_tensor_tensor(
                                r2[:, mo * BTOK + sl * 512: mo * BTOK + (sl + 1) * 512],
                                rps[:], bo[:, mo:mo + 1],
                                xt[:, mo * BTOK + sl * 512:
                                   mo * BTOK + (sl + 1) * 512].bitcast(F32),
                                ADD, ADD)
                    for dc in range(4):
                        nc.sync.dma_start(
                            r2d[dc, :, :].rearrange(
                                "p (t n) -> p t n", n=NS)[:, :, b * NB:(b + 1) * NB],
                            r2[:, dc * BTOK:(dc + 1) * BTOK].rearrange(
                                "p (t n) -> p t n", n=NB))

            # ---------------- Phase B2: LN + FFN (token-parallel) -----------
            with tc.tile_pool(name="wC", bufs=1) as wp, \
                 tc.tile_pool(name="actCx", bufs=2) as apx, \
                 tc.tile_pool(name="actC", bufs=1) as ap, \
                 tc.tile_pool(name="smC", bufs=3) as sc, \
                 tc.tile_pool(name="psC", bufs=2, space="PSUM") as pp, \
                 tc.tile_pool(name="psC1", bufs=1, space="PSUM") as pp1:
                f1t = _load_w(nc, wp, "f1", f1w, D, DFF, "f1")
                f2t = _load_w(nc, wp, "f2", f2w, DFF, D, "f2")
                b1 = wp.tile([128, 16], F32, tag="b1")
                nc.sync.dma_start(b1[:], f1b)
                b2 = wp.tile([128, 4], F32, tag="b2")
                nc.sync.dma_start(b2[:], f2b)
                g = wp.tile([128, 4], F32, tag="g")
                nc.sync.dma_start(g[:], lng)
                bb = wp.tile([128, 4], F32, tag="bb")
                nc.sync.dma_start(bb[:], lnb)

                for s in range(TOK // 512):
                    xt = apx.tile([128, 2048], F32R, tag="xt")
                    for dc in range(4):
                        nc.sync.dma_start(
                            xt[:, dc * 512:(dc + 1) * 512],
                            r2d[dc, :, s * 512:(s + 1) * 512].bitcast(F32R))
                    sps = pp1.tile([1, 512], F32, tag="st0")
                    sqps = pp1.tile([1, 512], F32, tag="st1")
                    for dc in range(4):
                        sq = sc.tile([128, 512], F32R, tag="sq")
                        nc.scalar.activation(
                            sq[:], xt[:, dc * 512:(dc + 1) * 512].bitcast(F32),
                            SQUARE)
                        _mm(nc, sps[:], ones_r128[:],
                            xt[:, dc * 512:(dc + 1) * 512], dc == 0, dc == 3)
                        _mm(nc, sqps[:], ones_r128[:], sq[:], dc == 0, dc == 3)
                    nmean = sc.tile([1, 512], F32, tag="nm")
                    nc.vector.tensor_scalar_mul(nmean[:], sps[:], -1.0 / D)
                    msq = sc.tile([1, 512], F32, tag="mq")
                    nc.vector.tensor_scalar_mul(msq[:], sqps[:], 1.0 / D)
                    m2 = sc.tile([1, 512], F32, tag="m2")
                    nc.vector.tensor_mul(m2[:], nmean[:], nmean[:])
                    var = sc.tile([1, 512], F32, tag="va")
                    nc.vector.tensor_sub(var[:], msq[:], m2[:])
                    nc.vector.tensor_scalar_add(var[:], var[:], 1e-5)
                    sd = sc.tile([1, 512], F32, tag="sd")
                    nc.scalar.activation(sd[:], var[:],
                                         mybir.ActivationFunctionType.Sqrt)
                    rstd = sc.tile([1, 512], F32R, tag="rs")
                    nc.vector.reciprocal(rstd[:], sd[:])
                    cneg = sc.tile([1, 512], F32R, tag="cn")
                    nc.vector.tensor_mul(cneg[:], nmean[:], rstd[:].bitcast(F32))
                    aps = pp1.tile([128, 512], F32, tag="bc0")
                    _mm(nc, aps[:], ones_r1[:], rstd[:], True, True)
                    cps = pp1.tile([128, 512], F32, tag="bc1")
                    _mm(nc, cps[:], ones_r1[:], cneg[:], True, True)
                    yt = ap.tile([128, 2048], F32R, tag="yt")
                    for dc in range(4):
                        u = sc.tile([128, 512], F32, tag="u")
                        nc.vector.tensor_mul(
                            u[:], xt[:, dc * 512:(dc + 1) * 512].bitcast(F32),
                            aps[:])
                        dt = sc.tile([128, 512], F32, tag="dt")
                        nc.vector.tensor_scalar(dt[:], cps[:], g[:, dc:dc + 1],
                                                bb[:, dc:dc + 1], MULT, ADD)
                        nc.vector.scalar_tensor_tensor(
                            yt[:, dc * 512:(dc + 1) * 512], u[:],
                            g[:, dc:dc + 1], dt[:], MULT, ADD)
                    ht = ap.tile([128, 16 * 512], F32R, tag="ht")
                    for mo in range(16):
                        hps = pp.tile([128, 512], F32, tag="mm0")
                        for dc in range(4):
                            _mm(nc, hps[:],
                                f1t[:, dc * DFF + mo * 128: dc * DFF + (mo + 1) * 128],
                                yt[:, dc * 512:(dc + 1) * 512], dc == 0, dc == 3)
                        nc.vector.tensor_scalar(
                            ht[:, mo * 512:(mo + 1) * 512], hps[:],
                            b1[:, mo:mo + 1], 0.0, ADD, MAX)
                    for mo in range(4):
                        ops2 = pp.tile([128, 512], F32, tag="mm1")
                        for dc in range(16):
                            _mm(nc, ops2[:],
                                f2t[:, dc * D + mo * 128: dc * D + (mo + 1) * 128],
                                ht[:, dc * 512:(dc + 1) * 512], dc == 0, dc == 15)
                        ou = sc.tile([128, 512], F32, tag="ou")
                        nc.vector.scalar_tensor_tensor(
                            ou[:], ops2[:], b2[:, mo:mo + 1],
                            xt[:, mo * 512:(mo + 1) * 512].bitcast(F32), ADD, ADD)
                        nc.sync.dma_start(
                            out[mo * 128:(mo + 1) * 128, s * 512:(s + 1) * 512],
                            ou[:])
    nc.compile()
    return nc


_CACHED = None
LAST_RESULT = None


def _get_nc():
    global _CACHED
    if _CACHED is None:
        _CACHED = build()
    return _CACHED


def kernel(**inputs):
    x = np.asarray(inputs["x"], np.float32)          # [T, N, D]
    base = {}
    for p in ("s", "t"):
        for m in ("q", "k", "v", "o"):
            wm = np.asarray(inputs[f"{p}{m}_w"], np.float32)
            bm = np.asarray(inputs[f"{p}{m}_b"], np.float32)
            if m == "q":                              # fold 1/sqrt(DK)
                wm, bm = wm / np.sqrt(DK), bm / np.sqrt(DK)
            base[p + m + "w"] = np.ascontiguousarray(wm)
            if m == "v":
                base[p + "vbb"] = np.ascontiguousarray(
                    np.broadcast_to(bm, (128, D)))
            else:
                base[p + m + "b"] = np.ascontiguousarray(
                    bm.reshape(4, 128).T)
    base["f1w"] = np.ascontiguousarray(np.asarray(inputs["f1_w"], np.float32))
    base["f1b"] = np.ascontiguousarray(
        np.asarray(inputs["f1_b"], np.float32).reshape(16, 128).T)
    base["f2w"] = np.ascontiguousarray(np.asarray(inputs["f2_w"], np.float32))
    base["f2b"] = np.ascontiguousarray(
        np.asarray(inputs["f2_b"], np.float32).reshape(4, 128).T)
    base["lng"] = np.ascontiguousarray(
        np.asarray(inputs["ln3_g"], np.float32).reshape(4, 128).T)
    base["lnb"] = np.ascontiguousarray(
        np.asarray(inputs["ln3_b"], np.float32).reshape(4, 128).T)
    base["ones1"] = np.ones((1, 128), np.float32)
    base["ones128"] = np.ones((128, 1), np.float32)

    in_maps = []
    for i in range(NC):
        m = dict(base)
        m["xs"] = np.ascontiguousarray(
            x[i * TS:(i + 1) * TS].reshape(TOK, D).T)
        in_maps.append(m)

    ncm = _get_nc()
    res = bass_utils.run_bass_kernel_spmd(
        ncm, in_maps, core_ids=list(range(NC)), trace=False)
    global LAST_RESULT
    LAST_RESULT = res
    o = np.stack([r["out"] for r in res.results])     # [8, D, TOK] f=(t*32+nl)
    return np.ascontiguousarray(
        o.reshape(NC, D, T, NS).transpose(2, 0, 3, 1).reshape(T, N, D))


# revision 12
# speedup vs baseline: 1.0296x; 1.0296x over previous
"""DecoderLayer (spatial attn + causal temporal attn + FFN) on 8 TRN2 NeuronCores.

Sharding: data-parallel. Spatial attention shards T (16 t-slices/core);
an on-device AllToAll reshards to N (32 cells/core) for causal temporal
attention + FFN. Activations are kept feature-major ([d, token]) on chip so
every matmul consumes natural layouts; the host pre/post-transposes x/out.

Matmul dtypes: fp32r (full-rate fp32 at free-dim>=256) for projections/FFN
and spatial scores; bf16 for softmax weights, V, and temporal scores.
Attention processes head PAIRS so softmax/normalize elementwise ops run at
the full 128-partition width.
"""
import sys

sys.path.insert(0, "/opt/trn_rl_repo")
import numpy as np
import concourse.bass as bass
import concourse.mybir as mybir
import concourse.tile as tile
from concourse import bacc
from concourse import bass_utils
from concourse.masks import make_upper_triangular

F32 = mybir.dt.float32
F32R = mybir.dt.float32r
BF16 = mybir.dt.bfloat16
EXP = mybir.ActivationFunctionType.Exp
SQUARE = mybir.ActivationFunctionType.Square
SQRT = mybir.ActivationFunctionType.Sqrt
IDENT = mybir.ActivationFunctionType.Identity
RELU = mybir.ActivationFunctionType.Relu
ADD = mybir.AluOpType.add
MULT = mybir.AluOpType.mult

T, N, D, H, DK, DFF = 128, 256, 512, 8, 64, 2048
NC = 8
TS = T // NC          # 16 t-slices per core (spatial)
NS = N // NC          # 32 cells per core (temporal)
TOK = TS * N          # 4096 tokens per core
NB = 8                # temporal batch: cells per batch
NBATCH = NS // NB     # 4 temporal batches
BTOK = T * NB         # 1024 tokens per temporal batch


def _mm(nc, out, lhsT, rhs, start, stop):
    nc.tensor.matmul(out, lhsT, rhs, start=start, stop=stop)


def _load_w(nc, sb, dram_ap, kdim, ndim, tag, dt=F32R):
    """Weight [kdim, ndim] -> SBUF [128, (kdim//128)*ndim], chunk dc at
    cols [dc*ndim, (dc+1)*ndim). Single multi-dim DMA."""
    kc = kdim // 128
    t = sb.tile([128, kc * ndim], dt, tag=tag, name=tag)
    nc.sync.dma_start(
        t[:], dram_ap.rearrange("(dc p) n -> p dc n", p=128).bitcast(dt))
    return t


def _attn_weights(nc, wp, w, p):
    wq = _load_w(nc, wp, w[p + "qw"], D, D, "wq")
    wk = _load_w(nc, wp, w[p + "kw"], D, D, "wk")
    wv = _load_w(nc, wp, w[p + "vw"], D, D, "wv")
    wo = _load_w(nc, wp, w[p + "ow"], D, D, "wo")
    bias = {}
    for m in ("q", "k", "o"):
        bt = wp.tile([128, 4], F32, tag="b" + m, name="b" + m)
        nc.sync.dma_start(bt[:], w[p + m + "b"])
        bias[m] = bt
    bvb = wp.tile([128, D], F32, tag="bvb", name="bvb")
    nc.sync.dma_start(bvb[:], w[p + "vbb"])
    return wq, wk, wv, wo, bias, bvb


def build():
    nc = bacc.Bacc("TRN2", target_bir_lowering=False, debug=False,
                   num_devices=NC)

    def inp(name, shape):
        return nc.dram_tensor(name, shape, F32, kind="ExternalInput").ap()

    xs = inp("xs", [D, TOK])                       # spatial shard, feature-major
    w = {}
    for p in ("s", "t"):
        for m in ("q", "k", "v", "o"):
            w[p + m + "w"] = inp(p + m + "w", [D, D])
        for m in ("q", "k", "o"):
            w[p + m + "b"] = inp(p + m + "b", [128, 4])
        w[p + "vbb"] = inp(p + "vbb", [128, D])
    f1w = inp("f1w", [D, DFF])
    f1b = inp("f1b", [128, DFF // 128])
    f2w = inp("f2w", [DFF, D])
    f2b = inp("f2b", [128, 4])
    lngb = inp("lngb", [2, D])                     # row0 = ln3_g, row1 = ln3_b
    ones512 = inp("ones512", [1, 512])
    ones128 = inp("ones128", [128, 1])
    out = nc.dram_tensor("out", [D, TOK], F32, kind="ExternalOutput").ap()

    with tile.TileContext(nc) as tc, \
         nc.allow_low_precision(reason="bf16 softmax weights and values"):
        with tc.tile_pool(name="const", bufs=1) as cb, \
             tc.tile_pool(name="dram", bufs=1, space="DRAM") as dram:
            ones_bf = cb.tile([128, 1], BF16, tag="obf")
            nc.vector.memset(ones_bf[:], 1.0)
            ones_r1 = cb.tile([1, 512], F32R, tag="o1r")
            nc.sync.dma_start(ones_r1[:], ones512.bitcast(F32R))
            ones_r128 = cb.tile([128, 1], F32R, tag="o128r")
            nc.sync.dma_start(ones_r128[:], ones128.bitcast(F32R))
            mask = cb.tile([128, 128], BF16, tag="mask")
            make_upper_triangular(nc, mask[:], val=1.0, diag=True)
            mask4 = cb.tile([128, 512], BF16, tag="mask4")
            for i in range(4):
                nc.vector.tensor_copy(mask4[:, i * 128:(i + 1) * 128], mask[:])
            lgb = cb.tile([2, D], F32R, tag="lgb")
            nc.sync.dma_start(lgb[:], lngb.bitcast(F32R))

            # a2a buffers: [dst, tloc(8), dc, p, n(32)] per half
            half_shape = [NC, TS // 2, 4, 128, 32]
            sends = [dram.tile(half_shape, F32, name=f"send{i}",
                               tag=f"send{i}") for i in range(2)]
            recvs = [dram.tile(half_shape, F32, name=f"recv{i}",
                               tag=f"recv{i}") for i in range(2)]
            r2d = dram.tile([4, 128, TOK], F32, name="r2d", tag="r2d")

            # ---------------- Phase A: spatial attention over n (shard T) ---
            with tc.tile_pool(name="wA", bufs=1) as wp, \
                 tc.tile_pool(name="actA", bufs=2) as ap, \
                 tc.tile_pool(name="smA", bufs=4) as sc, \
                 tc.tile_pool(name="psA", bufs=2, space="PSUM") as pp:
                wq, wk, wv, wo, bias, bvb = _attn_weights(nc, wp, w, "s")

                for tp in range(TS // 2):   # 8 pairs of t-slices, 512 tok each
                    xt = ap.tile([128, 2048], F32R, tag="xt")
                    nc.sync.dma_start(
                        xt[:], xs.rearrange("(dc p) f -> p dc f", p=128)
                        [:, :, tp * 512:(tp + 1) * 512].bitcast(F32R))
                    qt = ap.tile([128, 2048], F32R, tag="qt")
                    kt = ap.tile([128, 2048], F32R, tag="kt")
                    for mo in range(4):
                        qps = pp.tile([128, 512], F32, tag="pa")
                        kps = pp.tile([128, 512], F32, tag="pb")
                        for dc in range(4):
                            _mm(nc, qps[:],
                                wq[:, dc * D + mo * 128: dc * D + (mo + 1) * 128],
                                xt[:, dc * 512:(dc + 1) * 512], dc == 0, dc == 3)
                            _mm(nc, kps[:],
                                wk[:, dc * D + mo * 128: dc * D + (mo + 1) * 128],
                                xt[:, dc * 512:(dc + 1) * 512], dc == 0, dc == 3)
                        nc.scalar.activation(qt[:, mo * 512:(mo + 1) * 512],
                                             qps[:], IDENT,
                                             bias=bias["q"][:, mo:mo + 1])
                        nc.vector.tensor_scalar_add(
                            kt[:, mo * 512:(mo + 1) * 512], kps[:],
                            bias["k"][:, mo:mo + 1])
                    # V token-major, bf16: chunk c (128 tokens) at cols c*512..
                    vt = ap.tile([128, 2048], BF16, tag="vt")
                    for c in range(4):
                        vps = pp.tile([128, 512], F32, tag="pa")
                        for dc in range(4):
                            _mm(nc, vps[:],
                                xt[:, dc * 512 + c * 128: dc * 512 + (c + 1) * 128],
                                wv[:, dc * D:(dc + 1) * D], dc == 0, dc == 3)
                        nc.vector.tensor_add(vt[:, c * 512:(c + 1) * 512],
                                             vps[:], bvb[:])
                    ot = ap.tile([128, 2048], F32R, tag="ot")
                    for hp in range(4):           # head pair chunk
                        for hh in range(2):
                            h = 2 * hp + hh
                            po = 64 * hh
                            es_d = {}
                            dps = pp.tile([1, 512], F32, tag="pd")
                            for tl in range(2):
                                base = hp * 512 + tl * 256
                                sps = pp.tile([128, 512], F32, tag="pb")
                                for kc in range(2):
                                    _mm(nc, sps[:, kc * 256:(kc + 1) * 256],
                                        kt[po:po + 64,
                                           base + kc * 128: base + (kc + 1) * 128],
                                        qt[po:po + 64, base:base + 256],
                                        True, True)
                                es = sc.tile([128, 512], BF16, tag="es")
                                nc.scalar.activation(es[:], sps[:], EXP)
                                for kc in range(2):
                                    _mm(nc, dps[:, tl * 256:(tl + 1) * 256],
                                        ones_bf[:],
                                        es[:, kc * 256:(kc + 1) * 256],
                                        kc == 0, kc == 1)
                                es_d[tl] = es
                            rec = sc.tile([1, 512], F32R, tag="rec")
                            nc.vector.reciprocal(rec[:], dps[:])
                            bps = pp.tile([64, 512], F32, tag="pe")
                            _mm(nc, bps[:], ones_r1[:, :64], rec[:], True, True)
                            ops = pp.tile([64, 512], F32, tag="pa")
                            for tl in range(2):
                                for kc in range(2):
                                    _mm(nc, ops[:, tl * 256:(tl + 1) * 256],
                                        vt[:, (2 * tl + kc) * 512 + h * 64:
                                           (2 * tl + kc) * 512 + (h + 1) * 64],
                                        es_d[tl][:, kc * 256:(kc + 1) * 256],
                                        kc == 0, kc == 1)
                            osl = ot[po:po + 64, hp * 512:(hp + 1) * 512]
                            nc.scalar.copy(osl, ops[:])
                            nc.vector.tensor_mul(osl, osl, bps[:])
                    r1 = ap.tile([128, 2048], F32, tag="r1")
                    for mo in range(4):
                        rps = pp.tile([128, 512], F32, tag="pb")
                        for dc in range(4):
                            _mm(nc, rps[:],
                                wo[:, dc * D + mo * 128: dc * D + (mo + 1) * 128],
                                ot[:, dc * 512:(dc + 1) * 512], dc == 0, dc == 3)
                        nc.vector.scalar_tensor_tensor(
                            r1[:, mo * 512:(mo + 1) * 512], rps[:],
                            bias["o"][:, mo:mo + 1],
                            xt[:, mo * 512:(mo + 1) * 512].bitcast(F32),
                            ADD, ADD)
                    # scatter to a2a send buffers: 1 DMA per t-slice
                    r1v = r1.rearrange("p (dc tl j n) -> p tl dc j n",
                                       dc=4, tl=2, j=8)
                    for tl in range(2):
                        t = 2 * tp + tl
                        sh, tloc = t // 8, t % 8
                        for j in range(NC):
                            nc.sync.dma_start(
                                sends[sh][j, tloc].transpose([1, 0, 2]),
                                r1v[:, tl, :, j])
                    if tp == 3:
                        nc.gpsimd.collective_compute(
                            "AllToAll", mybir.AluOpType.bypass,
                            replica_groups=[list(range(NC))],
                            ins=[sends[0][:].opt()], outs=[recvs[0][:].opt()])
                nc.gpsimd.collective_compute(
                    "AllToAll", mybir.AluOpType.bypass,
                    replica_groups=[list(range(NC))],
                    ins=[sends[1][:].opt()], outs=[recvs[1][:].opt()])

            # ---------------- Phase B: temporal attention over t (shard N) --
            with tc.tile_pool(name="wB", bufs=1) as wp, \
                 tc.tile_pool(name="actBx", bufs=2) as apx, \
                 tc.tile_pool(name="actB", bufs=1) as ap, \
                 tc.tile_pool(name="smB", bufs=4) as sc, \
                 tc.tile_pool(name="psB", bufs=2, space="PSUM") as pp:
                wq, wk, wv, wo, bias, bvb = _attn_weights(nc, wp, w, "t")

                for b in range(NBATCH):   # 8 cells per batch, 1024 tokens
                    xt = apx.tile([128, 4 * BTOK], F32R, tag="xt")
                    xtv = xt.rearrange("p (dc s2 hf2 tl nl) -> p dc s2 hf2 tl nl",
                                       dc=4, s2=8, hf2=2, nl=NB)
                    for src in range(NC):
                        for hf in range(2):
                            for dc in range(4):
                                nc.sync.dma_start(
                                    xtv[:, dc, src, hf],
                                    recvs[hf][src, :, dc, :,
                                              b * NB:(b + 1) * NB]
                                    .transpose([1, 0, 2]).bitcast(F32R))
                    qt = ap.tile([128, 4 * BTOK], BF16, tag="qt")
                    kt = ap.tile([128, 4 * BTOK], BF16, tag="kt")
                    for mo in range(4):
                        for sl in range(2):
                            qps = pp.tile([128, 512], F32, tag="pa")
                            kps = pp.tile([128, 512], F32, tag="pb")
                            for dc in range(4):
                                _mm(nc, qps[:],
                                    wq[:, dc * D + mo * 128: dc * D + (mo + 1) * 128],
                                    xt[:, dc * BTOK + sl * 512:
                                       dc * BTOK + (sl + 1) * 512], dc == 0, dc == 3)
                                _mm(nc, kps[:],
                                    wk[:, dc * D + mo * 128: dc * D + (mo + 1) * 128],
                                    xt[:, dc * BTOK + sl * 512:
                                       dc * BTOK + (sl + 1) * 512], dc == 0, dc == 3)
                            nc.scalar.activation(
                                qt[:, mo * BTOK + sl * 512: mo * BTOK + (sl + 1) * 512],
                                qps[:], IDENT, bias=bias["q"][:, mo:mo + 1])
                            nc.vector.tensor_scalar_add(
                                kt[:, mo * BTOK + sl * 512: mo * BTOK + (sl + 1) * 512],
                                kps[:], bias["k"][:, mo:mo + 1])
                    vt = ap.tile([128, NB * 512], BF16, tag="vt")
                    for nl in range(NB):
                        vps = pp.tile([128, 512], F32, tag="pa")
                        for dc in range(4):
                            lhsT = xt[:, dc * BTOK:(dc + 1) * BTOK].rearrange(
                                "p (t n) -> p t n", n=NB)[:, :, nl]
                            _mm(nc, vps[:], lhsT, wv[:, dc * D:(dc + 1) * D],
                                dc == 0, dc == 3)
                        nc.vector.tensor_add(vt[:, nl * 512:(nl + 1) * 512],
                                             vps[:], bvb[:])
                    ot = ap.tile([128, 4 * BTOK], F32R, tag="ot")
                    for quad in range(NB // 4):
                        n1 = 4 * quad
                        for h in range(H):
                            po, dch = (h % 2) * 64, h // 2
                            qh = qt[:, dch * BTOK:(dch + 1) * BTOK].rearrange(
                                "p (t n) -> p t n", n=NB)
                            kh = kt[:, dch * BTOK:(dch + 1) * BTOK].rearrange(
                                "p (t n) -> p t n", n=NB)
                            # sps blocks: 4 cells of head h
                            sps = pp.tile([128, 512], F32, tag="pb")
                            for j in range(4):
                                _mm(nc, sps[:, j * 128:(j + 1) * 128],
                                    kh[po:po + 64, :, n1 + j],
                                    qh[po:po + 64, :, n1 + j], True, True)
                            es = sc.tile([128, 512], BF16, tag="es")
                            nc.scalar.activation(es[:], sps[:], EXP)
                            esm = sc.tile([128, 512], BF16, tag="esm")
                            nc.vector.tensor_mul(esm[:], es[:], mask4[:])
                            dps = pp.tile([1, 512], F32, tag="pd")
                            _mm(nc, dps[:], ones_bf[:], esm[:], True, True)
                            rec = sc.tile([1, 512], F32R, tag="rec")
                            nc.vector.reciprocal(rec[:], dps[:])
                            bps = pp.tile([64, 512], F32, tag="pe")
                            _mm(nc, bps[:], ones_r1[:, :64], rec[:], True, True)
                            ops = pp.tile([64, 512], F32, tag="pa")
                            for j in range(4):
                                _mm(nc, ops[:, j * 128:(j + 1) * 128],
                                    vt[:, (n1 + j) * 512 + h * 64:
                                       (n1 + j) * 512 + (h + 1) * 64],
                                    esm[:, j * 128:(j + 1) * 128], True, True)
                            osl = ot[:, dch * BTOK:(dch + 1) * BTOK].rearrange(
                                "p (t n) -> p n t", n=NB)[po:po + 64,
                                                          n1:n1 + 4, :]
                            nc.scalar.copy(osl, ops[:])
                            nc.vector.tensor_mul(osl, osl, bps[:])
                    r2 = ap.tile([128, 4 * BTOK], F32, tag="r2")
                    for mo in range(4):
                        for sl in range(2):
                            rps = pp.tile([128, 512], F32, tag="pb")
                            for dc in range(4):
                                _mm(nc, rps[:],
                                    wo[:, dc * D + mo * 128: dc * D + (mo + 1) * 128],
                                    ot[:, dc * BTOK + sl * 512:
                                       dc * BTOK + (sl + 1) * 512], dc == 0, dc == 3)
                            nc.vector.scalar_tensor_tensor(
                                r2[:, mo * BTOK + sl * 512: mo * BTOK + (sl + 1) * 512],
                                rps[:], bias["o"][:, mo:mo + 1],
                                xt[:, mo * BTOK + sl * 512:
                                   mo * BTOK + (sl + 1) * 512].bitcast(F32),
                                ADD, ADD)
                    r2v = r2.rearrange("p (dc t n) -> p dc t n", dc=4, n=NB)
                    for dc in range(4):
                        nc.sync.dma_start(
                            r2d[dc].rearrange("p (t n) -> p t n", n=NS)
                            [:, :, b * NB:(b + 1) * NB], r2v[:, dc])

            # ---------------- Phase B2: LN + FFN (token-parallel) -----------
            with tc.tile_pool(name="wC", bufs=1) as wp, \
                 tc.tile_pool(name="actCx", bufs=2) as apx, \
                 tc.tile_pool(name="actC", bufs=1) as ap, \
                 tc.tile_pool(name="smC", bufs=3) as sc, \
                 tc.tile_pool(name="psC", bufs=2, space="PSUM") as pp, \
                 tc.tile_pool(name="psC1", bufs=1, space="PSUM") as pp1:
                f1t = _load_w(nc, wp, f1w, D, DFF, "f1")
                f2t = _load_w(nc, wp, f2w, DFF, D, "f2")
                b1 = wp.tile([128, 16], F32, tag="b1")
                nc.sync.dma_start(b1[:], f1b)
                b2 = wp.tile([128, 4], F32, tag="b2")
                nc.sync.dma_start(b2[:], f2b)

                for s in range(TOK // 512):
                    xt = apx.tile([128, 2048], F32R, tag="xt")
                    nc.sync.dma_start(
                        xt[:], r2d[:, :, s * 512:(s + 1) * 512]
                        .transpose([1, 0, 2]).bitcast(F32R))
                    sps = pp1.tile([1, 512], F32, tag="st0")
                    sqps = pp1.tile([1, 512], F32, tag="st1")
                    for dc in range(4):
                        sq = sc.tile([128, 512], F32R, tag="sq")
                        nc.scalar.activation(
                            sq[:], xt[:, dc * 512:(dc + 1) * 512].bitcast(F32),
                            SQUARE)
                        _mm(nc, sps[:], ones_r128[:],
                            xt[:, dc * 512:(dc + 1) * 512], dc == 0, dc == 3)
                        _mm(nc, sqps[:], ones_r128[:], sq[:], dc == 0, dc == 3)
                    nmean = sc.tile([1, 512], F32, tag="nm")
                    nc.vector.tensor_scalar_mul(nmean[:], sps[:], -1.0 / D)
                    msq = sc.tile([1, 512], F32, tag="mq")
                    nc.vector.tensor_scalar_mul(msq[:], sqps[:], 1.0 / D)
                    m2 = sc.tile([1, 512], F32, tag="m2")
                    nc.vector.tensor_mul(m2[:], nmean[:], nmean[:])
                    var = sc.tile([1, 512], F32, tag="va")
                    nc.vector.tensor_sub(var[:], msq[:], m2[:])
                    nc.vector.tensor_scalar_add(var[:], var[:], 1e-5)
                    sd = sc.tile([1, 512], F32, tag="sd")
                    nc.scalar.activation(sd[:], var[:], SQRT)
                    rstd = sc.tile([1, 512], F32R, tag="rs")
                    nc.vector.reciprocal(rstd[:], sd[:])
                    cr = sc.tile([2, 512], F32R, tag="cr")
                    nc.vector.tensor_mul(cr[0:1, :], nmean[:],
                                         rstd[:].bitcast(F32))
                    nc.sync.dma_start(cr[1:2, :], ones512.bitcast(F32R))
                    # F = g (x) rstd ; E = g (x) cneg + b (x) 1   (broadcasts)
                    yt = ap.tile([128, 2048], F32R, tag="yt")
                    for dc in range(4):
                        fps = pp1.tile([128, 512], F32, tag="bc0")
                        _mm(nc, fps[:], lgb[0:1, dc * 128:(dc + 1) * 128],
                            rstd[:], True, True)
                        eps_ = pp1.tile([128, 512], F32, tag="bc1")
                        _mm(nc, eps_[:], lgb[:, dc * 128:(dc + 1) * 128],
                            cr[:], True, True)
                        u = sc.tile([128, 512], F32, tag="u")
                        nc.vector.tensor_mul(
                            u[:], xt[:, dc * 512:(dc + 1) * 512].bitcast(F32),
                            fps[:])
                        nc.vector.tensor_add(yt[:, dc * 512:(dc + 1) * 512],
                                             u[:], eps_[:])
                    ht = ap.tile([128, 16 * 512], F32R, tag="ht")
                    for mo in range(16):
                        hps = pp.tile([128, 512], F32, tag="mm0")
                        for dc in range(4):
                            _mm(nc, hps[:],
                                f1t[:, dc * DFF + mo * 128: dc * DFF + (mo + 1) * 128],
                                yt[:, dc * 512:(dc + 1) * 512], dc == 0, dc == 3)
                        nc.scalar.activation(ht[:, mo * 512:(mo + 1) * 512],
                                             hps[:], RELU,
                                             bias=b1[:, mo:mo + 1])
                    ou = sc.tile([128, 2048], F32, tag="ou")
                    for mo in range(4):
                        ops2 = pp.tile([128, 512], F32, tag="mm1")
                        for dc in range(16):
                            _mm(nc, ops2[:],
                                f2t[:, dc * D + mo * 128: dc * D + (mo + 1) * 128],
                                ht[:, dc * 512:(dc + 1) * 512], dc == 0, dc == 15)
                        nc.vector.scalar_tensor_tensor(
                            ou[:, mo * 512:(mo + 1) * 512], ops2[:],
                            b2[:, mo:mo + 1],
                            xt[:, mo * 512:(mo + 1) * 512].bitcast(F32),
                            ADD, ADD)
                    nc.sync.dma_start(
                        out.rearrange("(mo p) f -> p mo f", p=128)
                        [:, :, s * 512:(s + 1) * 512], ou[:])
    nc.compile()
    return nc


_CACHED = None
LAST_RESULT = None


def _get_nc():
    global _CACHED
    if _CACHED is None:
        _CACHED = build()
    return _CACHED


def kernel(**inputs):
    x = np.asarray(inputs["x"], np.float32)          # [T, N, D]
    base = {}
    for p in ("s", "t"):
        for m in ("q", "k", "v", "o"):
            wm = np.asarray(inputs[f"{p}{m}_w"], np.float32)
            bm = np.asarray(inputs[f"{p}{m}_b"], np.float32)
            if m == "q":                              # fold 1/sqrt(DK)
                wm, bm = wm / np.sqrt(DK), bm / np.sqrt(DK)
            base[p + m + "w"] = np.ascontiguousarray(wm)
            if m == "v":
                base[p + "vbb"] = np.ascontiguousarray(
                    np.broadcast_to(bm, (128, D)))
            else:
                base[p + m + "b"] = np.ascontiguousarray(
                    bm.reshape(4, 128).T)
    base["f1w"] = np.ascontiguousarray(np.asarray(inputs["f1_w"], np.float32))
    base["f1b"] = np.ascontiguousarray(
        np.asarray(inputs["f1_b"], np.float32).reshape(16, 128).T)
    base["f2w"] = np.ascontiguousarray(np.asarray(inputs["f2_w"], np.float32))
    base["f2b"] = np.ascontiguousarray(
        np.asarray(inputs["f2_b"], np.float32).reshape(4, 128).T)
    base["lngb"] = np.ascontiguousarray(np.stack([
        np.asarray(inputs["ln3_g"], np.float32),
        np.asarray(inputs["ln3_b"], np.float32)]))
    base["ones512"] = np.ones((1, 512), np.float32)
    base["ones128"] = np.ones((128, 1), np.float32)

    in_maps = []
    for i in range(NC):
        m = dict(base)
        m["xs"] = np.ascontiguousarray(
            x[i * TS:(i + 1) * TS].reshape(TOK, D).T)
        in_maps.append(m)

    ncm = _get_nc()
    res = bass_utils.run_bass_kernel_spmd(
        ncm, in_maps, core_ids=list(range(NC)), trace=False)
    global LAST_RESULT
    LAST_RESULT = res
    o = np.stack([r["out"] for r in res.results])     # [8, D, TOK] f=(t*32+nl)
    return np.ascontiguousarray(
        o.reshape(NC, D, T, NS).transpose(2, 0, 3, 1).reshape(T, N, D))
